# revision 22
# baseline (speedup 1.0000x reference)
"""Trainium2 Bass kernel for nn_FFEdgeCountingAutoencoder4.

Two-layer "edge counting" net. Per layer, each output node o picks an
operator (min/max) and a per-edge binary selection via hard Gumbel-softmax
with a fixed seed (jax key 42). Given the selections, the layer computes
    h[b,o] = min_i(mask? x : 1)   (min rows)
           = max_i(mask? x : 0)   (max rows)

The Gumbel draws depend only on the count tensors + a fixed key, so the
selection masks are computed on host (CPU jax, bit-exact threefry) and the
device does the O(B*out*in) masked reductions.

Device formulation (per output row o, with s=-1,c=1 for min rows else s=1,c=0):
    T[b,i]  = s*x[b,i] + c            (per-partition affine on ScalarE)
    g[b,o]  = max(0, max_i(T[b,i] + M[o,i]))   M = 0 selected / -100 masked
    h[b,o]  = s*g + c                 (same affine fixes up min rows: h = 1-g)
The masked max runs as TT-add + an in-place TT max tree (fp16 2x mode).

Sharding: data-parallel over batch, 16 rows per core; masks replicated.
"""

import numpy as np

B = 128
IN = 1024
HID = 512
NCORES = 8
BSH = B // NCORES  # 16

_PROGRAM_CACHE = {}


def _selection(otc, etc, li):
    """Host-side gumbel selection.

    Uses plain jax defaults (same backend/PRNG the reference runs under in
    this environment — the default PRNG here is backend-dependent rbg, so
    pinning to another device would produce different draws).
    """
    import jax
    import jax.numpy as jnp

    base = jax.random.key(42)
    k = jax.random.fold_in(base, li)
    k_op, k_edge = jax.random.split(k)
    otc_j = jnp.asarray(np.asarray(otc), dtype=jnp.float32)
    etc_j = jnp.asarray(np.asarray(etc), dtype=jnp.float32)
    g_op = jax.random.gumbel(k_op, otc_j.shape, dtype=jnp.float32)
    op_idx = jnp.argmax(otc_j + g_op, axis=-1)  # [out] 0=min 1=max
    g_e = jax.random.gumbel(k_edge, etc_j.shape, dtype=jnp.float32)
    sel_idx = jnp.argmax(etc_j + g_e, axis=-1)  # [out, n_ops, in]
    edge_sel = sel_idx[jnp.arange(etc_j.shape[0]), op_idx]  # [out, in]
    return np.asarray(op_idx == 0), np.asarray(edge_sel == 1)


def _build_program(debug=False):
    import concourse.bacc as bacc
    import concourse.mybir as mybir
    from concourse.tile import TileContext
    from concourse.masks import make_identity

    FP16 = mybir.dt.float16
    F32 = mybir.dt.float32
    AF = mybir.ActivationFunctionType
    ALU = mybir.AluOpType

    nc = bacc.Bacc("TRN2", target_bir_lowering=False, debug=False)
    x_d = nc.dram_tensor("x", [1, BSH * IN], FP16, kind="ExternalInput").ap()
    m0_d = nc.dram_tensor("m0", [HID, IN], FP16, kind="ExternalInput").ap()
    m1_d = nc.dram_tensor("m1", [IN, HID], FP16, kind="ExternalInput").ap()
    sb0_d = nc.dram_tensor("sb0", [128, 8], F32, kind="ExternalInput").ap()
    sb1_d = nc.dram_tensor("sb1", [128, 16], F32, kind="ExternalInput").ap()
    out_d = nc.dram_tensor("out", [BSH, IN], F32, kind="ExternalOutput").ap()
    if debug:
        dbg_hT = nc.dram_tensor("dbg_hT", [BSH, HID], FP16, kind="ExternalOutput").ap()
        dbg_h0 = nc.dram_tensor("dbg_h0", [128, 64], FP16, kind="ExternalOutput").ap()
        dbg_T = nc.dram_tensor("dbg_T", [128, 4096], FP16, kind="ExternalOutput").ap()

    NT0 = HID // 128  # 4 o-tiles in layer 0
    NT1 = IN // 128   # 8 o-tiles in layer 1

    with TileContext(nc) as tc:
        with (
            tc.tile_pool(name="const", bufs=1) as consts,
            tc.tile_pool(name="work", bufs=2) as work,
            tc.tile_pool(name="vpool", bufs=1) as vpool,
            tc.tile_pool(name="psum", bufs=1, space="PSUM") as pp,
        ):
            ones = consts.tile([1, 128], FP16, tag="ones")
            nc.gpsimd.memset(ones[:], 1.0)
            ident = consts.tile([128, 128], FP16, tag="ident")
            make_identity(nc, ident[:])

            xrow = consts.tile([1, BSH * IN], FP16, tag="x")
            nc.sync.dma_start(out=xrow[:], in_=x_d[:, :])
            m0_sb = consts.tile([128, NT0 * IN], FP16, tag="m0")
            for t in range(NT0):
                nc.sync.dma_start(
                    out=m0_sb[:, t * IN:(t + 1) * IN],
                    in_=m0_d[t * 128:(t + 1) * 128, :],
                )
            m1_sb = consts.tile([128, NT1 * HID], FP16, tag="m1")
            for t in range(NT1):
                nc.sync.dma_start(
                    out=m1_sb[:, t * HID:(t + 1) * HID],
                    in_=m1_d[t * 128:(t + 1) * 128, :],
                )
            sb0 = consts.tile([128, 8], F32, tag="sb0")
            nc.sync.dma_start(out=sb0[:], in_=sb0_d[:, :])
            sb1 = consts.tile([128, 16], F32, tag="sb1")
            nc.sync.dma_start(out=sb1[:], in_=sb1_d[:, :])

            h0 = consts.tile([128, NT0 * BSH], FP16, tag="h0")
            h0f = consts.tile([128, NT0 * BSH], FP16, tag="h0f")
            h1 = consts.tile([128, NT1 * BSH], FP16, tag="h1")
            h1f = consts.tile([128, NT1 * BSH], FP16, tag="h1f")
            hT = consts.tile([BSH, HID], FP16, tag="hT")
            hrow = consts.tile([1, BSH * HID], FP16, tag="hrow")
            out_sb = consts.tile([BSH, IN], F32, tag="out_sb")

            # ---------------- layer 0 ----------------
            # per o-tile: broadcast x into PSUM (4 batch rows at a time),
            # ACT affine -> T fp16, TT-add masks -> V, then a max tree over i
            for ot in range(NT0):
                V = vpool.tile([128, 16 * IN], FP16, tag="V")
                for g in range(4):
                    px = pp.tile([128, 4096], F32, tag="ps")
                    for bi in range(4):
                        b = g * 4 + bi
                        for mm in range(2):
                            off = b * IN + mm * 512
                            nc.tensor.matmul(
                                px[:, bi * 1024 + mm * 512: bi * 1024 + (mm + 1) * 512],
                                lhsT=ones[:],
                                rhs=xrow[0:1, off:off + 512],
                                start=True,
                                stop=True,
                            )
                    T = work.tile([128, 4096], FP16, tag="T")
                    for half in range(2):
                        nc.scalar.activation(
                            T[:, half * 2048:(half + 1) * 2048],
                            px[:, half * 2048:(half + 1) * 2048],
                            AF.Identity,
                            bias=sb0[:, 2 * ot + 1: 2 * ot + 2],
                            scale=sb0[:, 2 * ot: 2 * ot + 1],
                        )
                    for bi in range(4):
                        b = g * 4 + bi
                        nc.vector.tensor_tensor(
                            out=V[:, b * IN:(b + 1) * IN],
                            in0=T[:, bi * IN:(bi + 1) * IN],
                            in1=m0_sb[:, ot * IN:(ot + 1) * IN],
                            op=ALU.add,
                        )
                # in-place max tree over i: V viewed [128, 16, IN] -> [:, :, 0:1]
                lvl = V[:].rearrange("p (b i) -> p b i", b=BSH)
                n = IN
                while n > 2:
                    nc.vector.tensor_tensor(
                        out=lvl[:, :, 0:n // 2], in0=lvl[:, :, 0:n // 2],
                        in1=lvl[:, :, n // 2:n], op=ALU.max)
                    n //= 2
                # last level (n=2): reduce to [128, BSH] with clamp-to-0 done
                # by max against the zero-init h0? No: write max of the pair,
                # then max with 0 via tensor_scalar
                nc.vector.tensor_tensor(
                    out=h0[:, ot * BSH:(ot + 1) * BSH].rearrange("p (b i) -> p b i", b=BSH),
                    in0=lvl[:, :, 0:1], in1=lvl[:, :, 1:2], op=ALU.max)
                nc.vector.tensor_scalar_max(
                    h0[:, ot * BSH:(ot + 1) * BSH],
                    h0[:, ot * BSH:(ot + 1) * BSH], 0.0)
            for ot in range(NT0):
                nc.scalar.activation(
                    h0f[:, ot * BSH:(ot + 1) * BSH],
                    h0[:, ot * BSH:(ot + 1) * BSH],
                    AF.Identity,
                    bias=sb0[:, 2 * ot + 1: 2 * ot + 2],
                    scale=sb0[:, 2 * ot: 2 * ot + 1],
                )
            # h [512, 16] -> hT [16, 512] via TensorE transpose
            pt = pp.tile([BSH, HID], FP16, tag="ps")
            for ot in range(NT0):
                nc.tensor.transpose(
                    pt[:, ot * 128:(ot + 1) * 128],
                    h0f[:, ot * BSH:(ot + 1) * BSH],
                    ident[:],
                )
            nc.scalar.activation(hT[:], pt[:], AF.Copy)
            nc.sync.dma_start(out=hrow[:], in_=hT[:])
            if debug:
                nc.sync.dma_start(out=dbg_hT[:, :], in_=hT[:])
                nc.sync.dma_start(out=dbg_h0[:, :], in_=h0[:])

            # ---------------- layer 1 ----------------
            for ot in range(NT1):
                V2 = vpool.tile([128, 16 * HID], FP16, tag="V")
                for g in range(2):
                    ph = pp.tile([128, 4096], F32, tag="ps")
                    for bi in range(8):
                        b = g * 8 + bi
                        nc.tensor.matmul(
                            ph[:, bi * 512:(bi + 1) * 512],
                            lhsT=ones[:],
                            rhs=hrow[0:1, b * HID:(b + 1) * HID],
                            start=True,
                            stop=True,
                        )
                    T2 = work.tile([128, 4096], FP16, tag="T")
                    for half in range(2):
                        nc.scalar.activation(
                            T2[:, half * 2048:(half + 1) * 2048],
                            ph[:, half * 2048:(half + 1) * 2048],
                            AF.Identity,
                            bias=sb1[:, 2 * ot + 1: 2 * ot + 2],
                            scale=sb1[:, 2 * ot: 2 * ot + 1],
                        )
                    for bi in range(8):
                        b = g * 8 + bi
                        nc.vector.tensor_tensor(
                            out=V2[:, b * HID:(b + 1) * HID],
                            in0=T2[:, bi * HID:(bi + 1) * HID],
                            in1=m1_sb[:, ot * HID:(ot + 1) * HID],
                            op=ALU.add,
                        )
                lvl = V2[:].rearrange("p (b i) -> p b i", b=BSH)
                n = HID
                while n > 2:
                    nc.vector.tensor_tensor(
                        out=lvl[:, :, 0:n // 2], in0=lvl[:, :, 0:n // 2],
                        in1=lvl[:, :, n // 2:n], op=ALU.max)
                    n //= 2
                nc.vector.tensor_tensor(
                    out=h1[:, ot * BSH:(ot + 1) * BSH].rearrange("p (b i) -> p b i", b=BSH),
                    in0=lvl[:, :, 0:1], in1=lvl[:, :, 1:2], op=ALU.max)
                nc.vector.tensor_scalar_max(
                    h1[:, ot * BSH:(ot + 1) * BSH],
                    h1[:, ot * BSH:(ot + 1) * BSH], 0.0)
            for ot in range(NT1):
                nc.scalar.activation(
                    h1f[:, ot * BSH:(ot + 1) * BSH],
                    h1[:, ot * BSH:(ot + 1) * BSH],
                    AF.Identity,
                    bias=sb1[:, 2 * ot + 1: 2 * ot + 2],
                    scale=sb1[:, 2 * ot: 2 * ot + 1],
                )
            po = pp.tile([BSH, IN], FP16, tag="ps")
            for ot in range(NT1):
                nc.tensor.transpose(
                    po[:, ot * 128:(ot + 1) * 128],
                    h1f[:, ot * BSH:(ot + 1) * BSH],
                    ident[:],
                )
            nc.scalar.activation(out_sb[:], po[:], AF.Copy)
            nc.sync.dma_start(out=out_d[:, :], in_=out_sb[:])
    nc.finalize()
    return nc


def _prep_layer(opmin, mask):
    """Returns (madd fp16 [out, in], sb f32 [128, 2*ntiles])."""
    out_f = mask.shape[0]
    madd = np.where(mask, np.float16(0.0), np.float16(-100.0)).astype(np.float16)
    s = np.where(opmin, -1.0, 1.0).astype(np.float32)
    c = np.where(opmin, 1.0, 0.0).astype(np.float32)
    nt = out_f // 128
    sb = np.zeros((128, 2 * nt), np.float32)
    for t in range(nt):
        sb[:, 2 * t] = s[t * 128:(t + 1) * 128]
        sb[:, 2 * t + 1] = c[t * 128:(t + 1) * 128]
    return madd, sb


def kernel(**inputs):
    from concourse import bass_utils

    x = np.asarray(inputs["x"], dtype=np.float32)
    opmin0, mask0 = _selection(inputs["otc0"], inputs["etc0"], 0)
    opmin1, mask1 = _selection(inputs["otc1"], inputs["etc1"], 1)
    m0, sb0 = _prep_layer(opmin0, mask0)
    m1, sb1 = _prep_layer(opmin1, mask1)

    if "nc" not in _PROGRAM_CACHE:
        _PROGRAM_CACHE["nc"] = _build_program()
    nc = _PROGRAM_CACHE["nc"]

    x16 = x.astype(np.float16)
    in_maps = []
    for c in range(NCORES):
        in_maps.append({
            "x": x16[c * BSH:(c + 1) * BSH].reshape(1, -1),
            "m0": m0,
            "m1": m1,
            "sb0": sb0,
            "sb1": sb1,
        })
    res = bass_utils.run_bass_kernel_spmd(nc, in_maps, core_ids=list(range(NCORES)))
    out = np.concatenate([r["out"] for r in res.results], axis=0)
    return out.astype(np.float32)


# revision 23
# speedup vs baseline: 1.0497x; 1.0497x over previous
"""Trainium2 Bass kernel for nn_FFEdgeCountingAutoencoder4.

Two-layer "edge counting" net. Per layer, each output node o picks an
operator (min/max) and a per-edge binary selection via hard Gumbel-softmax
with a fixed seed (jax key 42). Given the selections, the layer computes
    h[b,o] = min_i(mask? x : 1)   (min rows)
           = max_i(mask? x : 0)   (max rows)

The Gumbel draws depend only on the count tensors + a fixed key, so the
selection masks are computed on host (CPU jax, bit-exact threefry) and the
device does the O(B*out*in) masked reductions.

Device formulation (per output row o, with s=-1,c=1 for min rows else s=1,c=0):
    T[b,i]  = s*x[b,i] + c            (per-partition affine on ScalarE)
    g[b,o]  = max(0, max_i(T[b,i] + M[o,i]))   M = 0 selected / -100 masked
    h[b,o]  = s*g + c                 (same affine fixes up min rows: h = 1-g)
The masked max runs as TT-add + an in-place TT max tree (fp16 2x mode).

Sharding: data-parallel over batch, 16 rows per core; masks replicated.
"""

import numpy as np

B = 128
IN = 1024
HID = 512
NCORES = 8
BSH = B // NCORES  # 16

_PROGRAM_CACHE = {}


def _selection(otc, etc, li):
    """Host-side gumbel selection.

    Uses plain jax defaults (same backend/PRNG the reference runs under in
    this environment — the default PRNG here is backend-dependent rbg, so
    pinning to another device would produce different draws).
    """
    import jax
    import jax.numpy as jnp

    base = jax.random.key(42)
    k = jax.random.fold_in(base, li)
    k_op, k_edge = jax.random.split(k)
    otc_j = jnp.asarray(np.asarray(otc), dtype=jnp.float32)
    etc_j = jnp.asarray(np.asarray(etc), dtype=jnp.float32)
    g_op = jax.random.gumbel(k_op, otc_j.shape, dtype=jnp.float32)
    op_idx = jnp.argmax(otc_j + g_op, axis=-1)  # [out] 0=min 1=max
    g_e = jax.random.gumbel(k_edge, etc_j.shape, dtype=jnp.float32)
    sel_idx = jnp.argmax(etc_j + g_e, axis=-1)  # [out, n_ops, in]
    edge_sel = sel_idx[jnp.arange(etc_j.shape[0]), op_idx]  # [out, in]
    return np.asarray(op_idx == 0), np.asarray(edge_sel == 1)


def _build_program(debug=False):
    import concourse.bacc as bacc
    import concourse.mybir as mybir
    from concourse.tile import TileContext
    from concourse.masks import make_identity

    FP16 = mybir.dt.float16
    F32 = mybir.dt.float32
    AF = mybir.ActivationFunctionType
    ALU = mybir.AluOpType

    nc = bacc.Bacc("TRN2", target_bir_lowering=False, debug=False)
    x_d = nc.dram_tensor("x", [1, BSH * IN], FP16, kind="ExternalInput").ap()
    m0_d = nc.dram_tensor("m0", [HID, IN], FP16, kind="ExternalInput").ap()
    m1_d = nc.dram_tensor("m1", [IN, HID], FP16, kind="ExternalInput").ap()
    sb0_d = nc.dram_tensor("sb0", [128, 8], F32, kind="ExternalInput").ap()
    sb1_d = nc.dram_tensor("sb1", [128, 16], F32, kind="ExternalInput").ap()
    out_d = nc.dram_tensor("out", [BSH, IN], F32, kind="ExternalOutput").ap()
    if debug:
        dbg_hT = nc.dram_tensor("dbg_hT", [BSH, HID], FP16, kind="ExternalOutput").ap()
        dbg_h0 = nc.dram_tensor("dbg_h0", [128, 64], FP16, kind="ExternalOutput").ap()
        dbg_T = nc.dram_tensor("dbg_T", [128, 4096], FP16, kind="ExternalOutput").ap()

    NT0 = HID // 128  # 4 o-tiles in layer 0
    NT1 = IN // 128   # 8 o-tiles in layer 1

    with TileContext(nc) as tc:
        with (
            tc.tile_pool(name="const", bufs=1) as consts,
            tc.tile_pool(name="work", bufs=2) as work,
            tc.tile_pool(name="vpool", bufs=2) as vpool,
            tc.tile_pool(name="psum", bufs=2, space="PSUM") as pp,
        ):
            ones = consts.tile([1, 128], FP16, tag="ones")
            nc.gpsimd.memset(ones[:], 1.0)
            ident = consts.tile([128, 128], FP16, tag="ident")
            make_identity(nc, ident[:])

            xrow = consts.tile([1, BSH * IN], FP16, tag="x")
            nc.sync.dma_start(out=xrow[:], in_=x_d[:, :])
            m0_sb = consts.tile([128, NT0 * IN], FP16, tag="m0")
            for t in range(NT0):
                nc.sync.dma_start(
                    out=m0_sb[:, t * IN:(t + 1) * IN],
                    in_=m0_d[t * 128:(t + 1) * 128, :],
                )
            m1_sb = consts.tile([128, NT1 * HID], FP16, tag="m1")
            for t in range(NT1):
                nc.sync.dma_start(
                    out=m1_sb[:, t * HID:(t + 1) * HID],
                    in_=m1_d[t * 128:(t + 1) * 128, :],
                )
            sb0 = consts.tile([128, 8], F32, tag="sb0")
            nc.sync.dma_start(out=sb0[:], in_=sb0_d[:, :])
            sb1 = consts.tile([128, 16], F32, tag="sb1")
            nc.sync.dma_start(out=sb1[:], in_=sb1_d[:, :])

            h0 = consts.tile([128, NT0 * BSH], FP16, tag="h0")
            h0f = consts.tile([128, NT0 * BSH], FP16, tag="h0f")
            h1 = consts.tile([128, NT1 * BSH], FP16, tag="h1")
            h1f = consts.tile([128, NT1 * BSH], FP16, tag="h1f")
            hT = consts.tile([BSH, HID], FP16, tag="hT")
            hrow = consts.tile([1, BSH * HID], FP16, tag="hrow")
            out_sb = consts.tile([BSH, IN], F32, tag="out_sb")

            # ---------------- layer 0 ----------------
            # per o-tile: broadcast x into PSUM (4 batch rows at a time),
            # ACT affine -> T fp16, TT-add masks -> V, then a max tree over i
            for ot in range(NT0):
                V = vpool.tile([128, 16 * IN], FP16, tag="V")
                for g in range(8):
                    px = pp.tile([128, 2048], F32, tag="ps")
                    for bi in range(2):
                        b = g * 2 + bi
                        for mm in range(2):
                            off = b * IN + mm * 512
                            nc.tensor.matmul(
                                px[:, bi * 1024 + mm * 512: bi * 1024 + (mm + 1) * 512],
                                lhsT=ones[:],
                                rhs=xrow[0:1, off:off + 512],
                                start=True,
                                stop=True,
                            )
                    T = work.tile([128, 2048], FP16, tag="T")
                    nc.scalar.activation(
                        T[:], px[:], AF.Identity,
                        bias=sb0[:, 2 * ot + 1: 2 * ot + 2],
                        scale=sb0[:, 2 * ot: 2 * ot + 1],
                    )
                    for bi in range(2):
                        b = g * 2 + bi
                        nc.vector.tensor_tensor(
                            out=V[:, b * IN:(b + 1) * IN],
                            in0=T[:, bi * IN:(bi + 1) * IN],
                            in1=m0_sb[:, ot * IN:(ot + 1) * IN],
                            op=ALU.add,
                        )
                # in-place max tree over i: V viewed [128, 16, IN] -> [:, :, 0:1]
                lvl = V[:].rearrange("p (b i) -> p b i", b=BSH)
                n = IN
                while n > 2:
                    nc.vector.tensor_tensor(
                        out=lvl[:, :, 0:n // 2], in0=lvl[:, :, 0:n // 2],
                        in1=lvl[:, :, n // 2:n], op=ALU.max)
                    n //= 2
                # last level (n=2): reduce to [128, BSH] with clamp-to-0 done
                # by max against the zero-init h0? No: write max of the pair,
                # then max with 0 via tensor_scalar
                nc.vector.tensor_tensor(
                    out=h0[:, ot * BSH:(ot + 1) * BSH].rearrange("p (b i) -> p b i", b=BSH),
                    in0=lvl[:, :, 0:1], in1=lvl[:, :, 1:2], op=ALU.max)
                nc.vector.tensor_scalar_max(
                    h0[:, ot * BSH:(ot + 1) * BSH],
                    h0[:, ot * BSH:(ot + 1) * BSH], 0.0)
            for ot in range(NT0):
                nc.scalar.activation(
                    h0f[:, ot * BSH:(ot + 1) * BSH],
                    h0[:, ot * BSH:(ot + 1) * BSH],
                    AF.Identity,
                    bias=sb0[:, 2 * ot + 1: 2 * ot + 2],
                    scale=sb0[:, 2 * ot: 2 * ot + 1],
                )
            # h [512, 16] -> hT [16, 512] via TensorE transpose
            pt = pp.tile([BSH, HID], FP16, tag="ps")
            for ot in range(NT0):
                nc.tensor.transpose(
                    pt[:, ot * 128:(ot + 1) * 128],
                    h0f[:, ot * BSH:(ot + 1) * BSH],
                    ident[:],
                )
            nc.scalar.activation(hT[:], pt[:], AF.Copy)
            nc.sync.dma_start(out=hrow[:], in_=hT[:])
            if debug:
                nc.sync.dma_start(out=dbg_hT[:, :], in_=hT[:])
                nc.sync.dma_start(out=dbg_h0[:, :], in_=h0[:])

            # ---------------- layer 1 ----------------
            for ot in range(NT1):
                V2 = vpool.tile([128, 16 * HID], FP16, tag="V")
                for g in range(4):
                    ph = pp.tile([128, 2048], F32, tag="ps")
                    for bi in range(4):
                        b = g * 4 + bi
                        nc.tensor.matmul(
                            ph[:, bi * 512:(bi + 1) * 512],
                            lhsT=ones[:],
                            rhs=hrow[0:1, b * HID:(b + 1) * HID],
                            start=True,
                            stop=True,
                        )
                    T2 = work.tile([128, 2048], FP16, tag="T")
                    nc.scalar.activation(
                        T2[:], ph[:], AF.Identity,
                        bias=sb1[:, 2 * ot + 1: 2 * ot + 2],
                        scale=sb1[:, 2 * ot: 2 * ot + 1],
                    )
                    for bi in range(4):
                        b = g * 4 + bi
                        nc.vector.tensor_tensor(
                            out=V2[:, b * HID:(b + 1) * HID],
                            in0=T2[:, bi * HID:(bi + 1) * HID],
                            in1=m1_sb[:, ot * HID:(ot + 1) * HID],
                            op=ALU.add,
                        )
                lvl = V2[:].rearrange("p (b i) -> p b i", b=BSH)
                n = HID
                while n > 2:
                    nc.vector.tensor_tensor(
                        out=lvl[:, :, 0:n // 2], in0=lvl[:, :, 0:n // 2],
                        in1=lvl[:, :, n // 2:n], op=ALU.max)
                    n //= 2
                nc.vector.tensor_tensor(
                    out=h1[:, ot * BSH:(ot + 1) * BSH].rearrange("p (b i) -> p b i", b=BSH),
                    in0=lvl[:, :, 0:1], in1=lvl[:, :, 1:2], op=ALU.max)
                nc.vector.tensor_scalar_max(
                    h1[:, ot * BSH:(ot + 1) * BSH],
                    h1[:, ot * BSH:(ot + 1) * BSH], 0.0)
            for ot in range(NT1):
                nc.scalar.activation(
                    h1f[:, ot * BSH:(ot + 1) * BSH],
                    h1[:, ot * BSH:(ot + 1) * BSH],
                    AF.Identity,
                    bias=sb1[:, 2 * ot + 1: 2 * ot + 2],
                    scale=sb1[:, 2 * ot: 2 * ot + 1],
                )
            po = pp.tile([BSH, IN], FP16, tag="ps")
            for ot in range(NT1):
                nc.tensor.transpose(
                    po[:, ot * 128:(ot + 1) * 128],
                    h1f[:, ot * BSH:(ot + 1) * BSH],
                    ident[:],
                )
            nc.scalar.activation(out_sb[:], po[:], AF.Copy)
            nc.sync.dma_start(out=out_d[:, :], in_=out_sb[:])
    nc.finalize()
    return nc


def _prep_layer(opmin, mask):
    """Returns (madd fp16 [out, in], sb f32 [128, 2*ntiles])."""
    out_f = mask.shape[0]
    madd = np.where(mask, np.float16(0.0), np.float16(-100.0)).astype(np.float16)
    s = np.where(opmin, -1.0, 1.0).astype(np.float32)
    c = np.where(opmin, 1.0, 0.0).astype(np.float32)
    nt = out_f // 128
    sb = np.zeros((128, 2 * nt), np.float32)
    for t in range(nt):
        sb[:, 2 * t] = s[t * 128:(t + 1) * 128]
        sb[:, 2 * t + 1] = c[t * 128:(t + 1) * 128]
    return madd, sb


def kernel(**inputs):
    from concourse import bass_utils

    x = np.asarray(inputs["x"], dtype=np.float32)
    opmin0, mask0 = _selection(inputs["otc0"], inputs["etc0"], 0)
    opmin1, mask1 = _selection(inputs["otc1"], inputs["etc1"], 1)
    m0, sb0 = _prep_layer(opmin0, mask0)
    m1, sb1 = _prep_layer(opmin1, mask1)

    if "nc" not in _PROGRAM_CACHE:
        _PROGRAM_CACHE["nc"] = _build_program()
    nc = _PROGRAM_CACHE["nc"]

    x16 = x.astype(np.float16)
    in_maps = []
    for c in range(NCORES):
        in_maps.append({
            "x": x16[c * BSH:(c + 1) * BSH].reshape(1, -1),
            "m0": m0,
            "m1": m1,
            "sb0": sb0,
            "sb1": sb1,
        })
    res = bass_utils.run_bass_kernel_spmd(nc, in_maps, core_ids=list(range(NCORES)))
    out = np.concatenate([r["out"] for r in res.results], axis=0)
    return out.astype(np.float32)


# revision 26
# speedup vs baseline: 1.2677x; 1.2077x over previous
"""Trainium2 Bass kernel for nn_FFEdgeCountingAutoencoder4.

Two-layer "edge counting" net. Per layer, each output node o picks an
operator (min/max) and a per-edge binary selection via hard Gumbel-softmax
with a fixed seed (jax key 42). Given the selections, the layer computes
    h[b,o] = min_i(mask? x : 1)   (min rows)
           = max_i(mask? x : 0)   (max rows)

The Gumbel draws depend only on the count tensors + the fixed key, so the
selection masks are computed on host (same jax defaults as the reference)
and the device does the O(B*out*in) masked reductions.

Device formulation (per output row o, with s=-1,c=1 for min rows else
s=1,c=0):
    v[b,o,i] = s[o]*x[b,i] + c[o] + M[o,i]      M = 0 selected / -100 masked
    g[b,o]   = max(0, max_i v[b,o,i])
    h[b,o]   = s[o]*g + c[o]

v is produced entirely on the TensorEngine: one matmul per (o-tile,
batch-group, i-chunk) whose stationary matrix packs 127 rows of
(c[o]+M[o,i]) plus a row of s[o], and whose moving matrix packs the
delta-selector pattern plus a row of x values. ScalarE copies PSUM->SBUF
fp16; VectorE runs an in-place pairwise max tree (2x fp16 mode).

Sharding: data-parallel over batch, 16 rows per core; weights replicated.
"""

import numpy as np

B = 128
IN = 1024
HID = 512
NCORES = 8
BSH = B // NCORES  # 16
CL = 127           # i-chunk length (127 mask rows + 1 x row per matmul)
NCH0 = (IN + CL - 1) // CL    # 9 chunks in layer 0
NCH1 = (HID + CL - 1) // CL   # 5 chunks in layer 1
BG0 = 2   # batch rows per PSUM group, layer 0 (2*1024 f32 = half of PSUM)
BG1 = 4   # batch rows per PSUM group, layer 1 (4*512 f32)

_PROGRAM_CACHE = {}


def _selection(otc, etc, li):
    """Host-side gumbel selection.

    Uses plain jax defaults (same backend/PRNG the reference runs under in
    this environment — the default PRNG here is backend-dependent rbg, so
    pinning to another device would produce different draws).
    """
    import jax
    import jax.numpy as jnp

    base = jax.random.key(42)
    k = jax.random.fold_in(base, li)
    k_op, k_edge = jax.random.split(k)
    otc_j = jnp.asarray(np.asarray(otc), dtype=jnp.float32)
    etc_j = jnp.asarray(np.asarray(etc), dtype=jnp.float32)
    g_op = jax.random.gumbel(k_op, otc_j.shape, dtype=jnp.float32)
    op_idx = jnp.argmax(otc_j + g_op, axis=-1)  # [out] 0=min 1=max
    g_e = jax.random.gumbel(k_edge, etc_j.shape, dtype=jnp.float32)
    sel_idx = jnp.argmax(etc_j + g_e, axis=-1)  # [out, n_ops, in]
    edge_sel = sel_idx[jnp.arange(etc_j.shape[0]), op_idx]  # [out, in]
    return np.asarray(op_idx == 0), np.asarray(edge_sel == 1)


def _prep_layer(opmin, mask):
    """Injection weights + affine vectors for one layer.

    Returns:
      W  [128, nch*out_f] fp16 — stationary matrices: W[i', k*out_f + o] =
         c[o] + M[o, k*CL + i'] for i' < len_k, W[127, .] = s[o], else 0.
      sel[128, bg*in_f]  fp16 — delta-selector moving matrix: sel[i', b*in_f
         + i] = (i' == i % CL); row 127 zeroed (x values DMA'd in at runtime).
      sb [128, 2*ntiles] f32 — per-o-tile (scale, bias) = (s, c) columns.
    """
    out_f, in_f = mask.shape
    nch = (in_f + CL - 1) // CL
    s = np.where(opmin, -1.0, 1.0).astype(np.float32)
    c = np.where(opmin, 1.0, 0.0).astype(np.float32)
    M = np.where(mask, 0.0, -100.0).astype(np.float32)
    Mp = c[:, None] + M  # [out, in]

    W = np.zeros((128, nch * out_f), np.float16)
    for k in range(nch):
        i0, i1 = k * CL, min((k + 1) * CL, in_f)
        W[0:i1 - i0, k * out_f:(k + 1) * out_f] = Mp[:, i0:i1].T
        W[127, k * out_f:(k + 1) * out_f] = s

    bg = BG0 if in_f == IN else BG1
    sel = np.zeros((128, bg * in_f), np.float16)
    i_idx = np.arange(in_f)
    j_idx = i_idx % CL  # selector row for column i
    for b in range(bg):
        sel[j_idx, b * in_f + i_idx] = 1.0

    nt = out_f // 128
    sb = np.zeros((128, 2 * nt), np.float32)
    for t in range(nt):
        sb[:, 2 * t] = s[t * 128:(t + 1) * 128]
        sb[:, 2 * t + 1] = c[t * 128:(t + 1) * 128]
    return W, sel, sb


def _build_program(debug=False):
    import concourse.bacc as bacc
    import concourse.mybir as mybir
    from concourse.tile import TileContext
    from concourse.masks import make_identity

    FP16 = mybir.dt.float16
    F32 = mybir.dt.float32
    AF = mybir.ActivationFunctionType
    ALU = mybir.AluOpType

    nc = bacc.Bacc("TRN2", target_bir_lowering=False, debug=False)
    x_d = nc.dram_tensor("x", [1, BSH * IN], FP16, kind="ExternalInput").ap()
    w0_d = nc.dram_tensor("w0", [128, NCH0 * HID], FP16, kind="ExternalInput").ap()
    w1_d = nc.dram_tensor("w1", [128, NCH1 * IN], FP16, kind="ExternalInput").ap()
    sel0_d = nc.dram_tensor("sel0", [128, BG0 * IN], FP16, kind="ExternalInput").ap()
    sel1_d = nc.dram_tensor("sel1", [128, BG1 * HID], FP16, kind="ExternalInput").ap()
    sb0_d = nc.dram_tensor("sb0", [128, 8], F32, kind="ExternalInput").ap()
    sb1_d = nc.dram_tensor("sb1", [128, 16], F32, kind="ExternalInput").ap()
    out_d = nc.dram_tensor("out", [BSH, IN], F32, kind="ExternalOutput").ap()
    if debug:
        dbg_hT = nc.dram_tensor("dbg_hT", [BSH, HID], FP16, kind="ExternalOutput").ap()
        dbg_h0 = nc.dram_tensor("dbg_h0", [128, 64], FP16, kind="ExternalOutput").ap()

    NT0 = HID // 128  # 4 o-tiles in layer 0
    NT1 = IN // 128   # 8 o-tiles in layer 1
    NG0 = BSH // BG0  # 8 batch groups in layer 0
    NG1 = BSH // BG1  # 4 batch groups in layer 1

    with TileContext(nc) as tc:
        with (
            tc.tile_pool(name="const", bufs=1) as consts,
            tc.tile_pool(name="vpool", bufs=2) as vpool,
            tc.tile_pool(name="psum", bufs=2, space="PSUM") as pp,
        ):
            ident = consts.tile([128, 128], FP16, tag="ident")
            make_identity(nc, ident[:])

            xrow = consts.tile([1, BSH * IN], FP16, tag="x")
            nc.sync.dma_start(out=xrow[:], in_=x_d[:, :])
            w0_sb = consts.tile([128, NCH0 * HID], FP16, tag="w0")
            nc.sync.dma_start(out=w0_sb[:], in_=w0_d[:, :])
            w1_sb = consts.tile([128, NCH1 * IN], FP16, tag="w1")
            nc.sync.dma_start(out=w1_sb[:], in_=w1_d[:, :])
            # two alternating moving-matrix sets per layer (row 127 carries
            # the per-group x values, rewritten while the other set runs)
            rhs0_a = consts.tile([128, BG0 * IN], FP16, tag="rhs0a")
            rhs0_b = consts.tile([128, BG0 * IN], FP16, tag="rhs0b")
            rhs1_a = consts.tile([128, BG1 * HID], FP16, tag="rhs1a")
            rhs1_b = consts.tile([128, BG1 * HID], FP16, tag="rhs1b")
            rhs0 = [rhs0_a, rhs0_b]
            rhs1 = [rhs1_a, rhs1_b]
            for s in range(2):
                nc.sync.dma_start(out=rhs0[s][:], in_=sel0_d[:, :])
                nc.sync.dma_start(out=rhs1[s][:], in_=sel1_d[:, :])
            sb0 = consts.tile([128, 8], F32, tag="sb0")
            nc.sync.dma_start(out=sb0[:], in_=sb0_d[:, :])
            sb1 = consts.tile([128, 16], F32, tag="sb1")
            nc.sync.dma_start(out=sb1[:], in_=sb1_d[:, :])

            h0 = consts.tile([128, NT0 * BSH], FP16, tag="h0")
            h0f = consts.tile([128, NT0 * BSH], FP16, tag="h0f")
            h1 = consts.tile([128, NT1 * BSH], FP16, tag="h1")
            h1f = consts.tile([128, NT1 * BSH], FP16, tag="h1f")
            hT = consts.tile([BSH, HID], FP16, tag="hT")
            hrow = consts.tile([1, BSH * HID], FP16, tag="hrow")
            out_sb = consts.tile([BSH, IN], F32, tag="out_sb")

            def layer(nt, ng, bg, in_f, nch, rhs_sets, xsrc, w_sb, h_out, out_f):
                for ot in range(nt):
                    V = vpool.tile([128, BSH * in_f], FP16, tag="V")
                    for g in range(ng):
                        rset = rhs_sets[g % 2]
                        # row 127 <- x values for this batch group
                        nc.sync.dma_start(
                            out=rset[127:128, :],
                            in_=xsrc[0:1, g * bg * in_f:(g + 1) * bg * in_f],
                        )
                        px = pp.tile([128, bg * in_f], F32, tag="ps")
                        pxv = px[:].rearrange("p (b i) -> p b i", b=bg)
                        rv = rset[:].rearrange("p (b i) -> p b i", b=bg)
                        for k in range(nch):
                            i0, i1 = k * CL, min((k + 1) * CL, in_f)
                            lhsT = w_sb[:, k * out_f + ot * 128:
                                        k * out_f + ot * 128 + 128]
                            # one matmul per (batch row, PSUM-bank segment):
                            # a matmul output may not cross a 512-f32 bank
                            for bi in range(bg):
                                p0 = i0
                                while p0 < i1:
                                    p1 = min(i1, (p0 // 512 + 1) * 512)
                                    nc.tensor.matmul(
                                        pxv[:, bi:bi + 1, p0:p1],
                                        lhsT=lhsT,
                                        rhs=rv[:, bi:bi + 1, p0:p1],
                                        start=True,
                                        stop=True,
                                    )
                                    p0 = p1
                        nc.scalar.activation(
                            V[:, g * bg * in_f:(g + 1) * bg * in_f], px[:],
                            AF.Copy)
                    # in-place max tree over i: V [128, BSH, in_f] -> [:, :, 0:1]
                    lvl = V[:].rearrange("p (b i) -> p b i", b=BSH)
                    n = in_f
                    while n > 2:
                        nc.vector.tensor_tensor(
                            out=lvl[:, :, 0:n // 2], in0=lvl[:, :, 0:n // 2],
                            in1=lvl[:, :, n // 2:n], op=ALU.max)
                        n //= 2
                    nc.vector.tensor_tensor(
                        out=h_out[:, ot * BSH:(ot + 1) * BSH].rearrange(
                            "p (b i) -> p b i", b=BSH),
                        in0=lvl[:, :, 0:1], in1=lvl[:, :, 1:2], op=ALU.max)
                    nc.vector.tensor_scalar_max(
                        h_out[:, ot * BSH:(ot + 1) * BSH],
                        h_out[:, ot * BSH:(ot + 1) * BSH], 0.0)

            # ---------------- layer 0 ----------------
            layer(NT0, NG0, BG0, IN, NCH0, rhs0, xrow, w0_sb, h0, HID)
            for ot in range(NT0):
                nc.scalar.activation(
                    h0f[:, ot * BSH:(ot + 1) * BSH],
                    h0[:, ot * BSH:(ot + 1) * BSH],
                    AF.Identity,
                    bias=sb0[:, 2 * ot + 1: 2 * ot + 2],
                    scale=sb0[:, 2 * ot: 2 * ot + 1],
                )
            # h [512, 16] -> hT [16, 512] -> hrow [1, 8192]
            pt = pp.tile([BSH, HID], FP16, tag="ps")
            for ot in range(NT0):
                nc.tensor.transpose(
                    pt[:, ot * 128:(ot + 1) * 128],
                    h0f[:, ot * BSH:(ot + 1) * BSH],
                    ident[:],
                )
            nc.scalar.activation(hT[:], pt[:], AF.Copy)
            nc.sync.dma_start(out=hrow[:], in_=hT[:])
            if debug:
                nc.sync.dma_start(out=dbg_hT[:, :], in_=hT[:])
                nc.sync.dma_start(out=dbg_h0[:, :], in_=h0[:])

            # ---------------- layer 1 ----------------
            layer(NT1, NG1, BG1, HID, NCH1, rhs1, hrow, w1_sb, h1, IN)
            for ot in range(NT1):
                nc.scalar.activation(
                    h1f[:, ot * BSH:(ot + 1) * BSH],
                    h1[:, ot * BSH:(ot + 1) * BSH],
                    AF.Identity,
                    bias=sb1[:, 2 * ot + 1: 2 * ot + 2],
                    scale=sb1[:, 2 * ot: 2 * ot + 1],
                )
            po = pp.tile([BSH, IN], FP16, tag="ps")
            for ot in range(NT1):
                nc.tensor.transpose(
                    po[:, ot * 128:(ot + 1) * 128],
                    h1f[:, ot * BSH:(ot + 1) * BSH],
                    ident[:],
                )
            nc.scalar.activation(out_sb[:], po[:], AF.Copy)
            nc.sync.dma_start(out=out_d[:, :], in_=out_sb[:])
    nc.finalize()
    return nc


def kernel(**inputs):
    from concourse import bass_utils

    x = np.asarray(inputs["x"], dtype=np.float32)
    opmin0, mask0 = _selection(inputs["otc0"], inputs["etc0"], 0)
    opmin1, mask1 = _selection(inputs["otc1"], inputs["etc1"], 1)
    w0, sel0, sb0 = _prep_layer(opmin0, mask0)
    w1, sel1, sb1 = _prep_layer(opmin1, mask1)

    if "nc" not in _PROGRAM_CACHE:
        _PROGRAM_CACHE["nc"] = _build_program()
    nc = _PROGRAM_CACHE["nc"]

    x16 = x.astype(np.float16)
    in_maps = []
    for c in range(NCORES):
        in_maps.append({
            "x": x16[c * BSH:(c + 1) * BSH].reshape(1, -1),
            "w0": w0,
            "w1": w1,
            "sel0": sel0,
            "sel1": sel1,
            "sb0": sb0,
            "sb1": sb1,
        })
    res = bass_utils.run_bass_kernel_spmd(nc, in_maps, core_ids=list(range(NCORES)))
    out = np.concatenate([r["out"] for r in res.results], axis=0)
    return out.astype(np.float32)


# revision 28
# speedup vs baseline: 1.3143x; 1.0367x over previous
"""Trainium2 Bass kernel for nn_FFEdgeCountingAutoencoder4.

Two-layer "edge counting" net. Per layer, each output node o picks an
operator (min/max) and a per-edge binary selection via hard Gumbel-softmax
with a fixed seed (jax key 42). Given the selections, the layer computes
    h[b,o] = min_i(mask? x : 1)   (min rows)
           = max_i(mask? x : 0)   (max rows)

The Gumbel draws depend only on the count tensors + the fixed key, so the
selection masks are computed on host (same jax defaults as the reference)
and the device does the O(B*out*in) masked reductions.

Device formulation (per output row o, with s=-1,c=1 for min rows else
s=1,c=0):
    v[b,o,i] = s[o]*x[b,i] + c[o] + M[o,i]      M = 0 selected / -100 masked
    g[b,o]   = max(0, max_i v[b,o,i])
    h[b,o]   = s[o]*g + c[o]

v is produced entirely on the TensorEngine: one matmul per (o-tile,
batch-group, i-chunk) whose stationary matrix packs 127 rows of
(c[o]+M[o,i]) plus a row of s[o], and whose moving matrix packs the
delta-selector pattern plus a row of x values. ScalarE copies PSUM->SBUF
fp16; VectorE runs an in-place pairwise max tree (2x fp16 mode).

Sharding: data-parallel over batch, 16 rows per core; weights replicated.
"""

import numpy as np

B = 128
IN = 1024
HID = 512
NCORES = 8
BSH = B // NCORES  # 16
CL = 127           # i-chunk length (127 mask rows + 1 x row per matmul)
NCH0 = (IN + CL - 1) // CL    # 9 chunks in layer 0
NCH1 = (HID + CL - 1) // CL   # 5 chunks in layer 1
BG0 = 2   # batch rows per PSUM group, layer 0 (2*1024 f32 = half of PSUM)
BG1 = 4   # batch rows per PSUM group, layer 1 (4*512 f32)

_PROGRAM_CACHE = {}


def _selection(otc, etc, li):
    """Host-side gumbel selection.

    Uses plain jax defaults (same backend/PRNG the reference runs under in
    this environment — the default PRNG here is backend-dependent rbg, so
    pinning to another device would produce different draws).
    """
    import jax
    import jax.numpy as jnp

    base = jax.random.key(42)
    k = jax.random.fold_in(base, li)
    k_op, k_edge = jax.random.split(k)
    otc_j = jnp.asarray(np.asarray(otc), dtype=jnp.float32)
    etc_j = jnp.asarray(np.asarray(etc), dtype=jnp.float32)
    g_op = jax.random.gumbel(k_op, otc_j.shape, dtype=jnp.float32)
    op_idx = jnp.argmax(otc_j + g_op, axis=-1)  # [out] 0=min 1=max
    g_e = jax.random.gumbel(k_edge, etc_j.shape, dtype=jnp.float32)
    sel_idx = jnp.argmax(etc_j + g_e, axis=-1)  # [out, n_ops, in]
    edge_sel = sel_idx[jnp.arange(etc_j.shape[0]), op_idx]  # [out, in]
    return np.asarray(op_idx == 0), np.asarray(edge_sel == 1)


def _prep_layer(opmin, mask):
    """Injection weights + affine vectors for one layer.

    Returns:
      W  [128, nch*out_f] fp16 — stationary matrices: W[i', k*out_f + o] =
         c[o] + M[o, k*CL + i'] for i' < len_k, W[127, .] = s[o], else 0.
      sel[128, bg*in_f]  fp16 — delta-selector moving matrix: sel[i', b*in_f
         + i] = (i' == i % CL); row 127 zeroed (x values DMA'd in at runtime).
      sb [128, 2*ntiles] f32 — per-o-tile (scale, bias) = (s, c) columns.
    """
    out_f, in_f = mask.shape
    nch = (in_f + CL - 1) // CL
    s = np.where(opmin, -1.0, 1.0).astype(np.float32)
    c = np.where(opmin, 1.0, 0.0).astype(np.float32)
    M = np.where(mask, 0.0, -100.0).astype(np.float32)
    Mp = c[:, None] + M  # [out, in]

    W = np.zeros((128, nch * out_f), np.float16)
    for k in range(nch):
        i0, i1 = k * CL, min((k + 1) * CL, in_f)
        W[0:i1 - i0, k * out_f:(k + 1) * out_f] = Mp[:, i0:i1].T
        W[127, k * out_f:(k + 1) * out_f] = s

    bg = BG0 if in_f == IN else BG1
    sel = np.zeros((128, bg * in_f), np.float16)
    i_idx = np.arange(in_f)
    j_idx = i_idx % CL  # selector row for column i
    for b in range(bg):
        sel[j_idx, b * in_f + i_idx] = 1.0

    nt = out_f // 128
    sb = np.zeros((128, 2 * nt), np.float32)
    for t in range(nt):
        sb[:, 2 * t] = s[t * 128:(t + 1) * 128]
        sb[:, 2 * t + 1] = c[t * 128:(t + 1) * 128]
    return W, sel, sb


def _build_program(debug=False):
    import concourse.bacc as bacc
    import concourse.mybir as mybir
    from concourse.tile import TileContext
    from concourse.masks import make_identity

    FP16 = mybir.dt.float16
    F32 = mybir.dt.float32
    AF = mybir.ActivationFunctionType
    ALU = mybir.AluOpType

    nc = bacc.Bacc("TRN2", target_bir_lowering=False, debug=False)
    x_d = nc.dram_tensor("x", [1, BSH * IN], FP16, kind="ExternalInput").ap()
    w0_d = nc.dram_tensor("w0", [128, NCH0 * HID], FP16, kind="ExternalInput").ap()
    w1_d = nc.dram_tensor("w1", [128, NCH1 * IN], FP16, kind="ExternalInput").ap()
    sel0_d = nc.dram_tensor("sel0", [128, BG0 * IN], FP16, kind="ExternalInput").ap()
    sel1_d = nc.dram_tensor("sel1", [128, BG1 * HID], FP16, kind="ExternalInput").ap()
    sb0_d = nc.dram_tensor("sb0", [128, 8], F32, kind="ExternalInput").ap()
    sb1_d = nc.dram_tensor("sb1", [128, 16], F32, kind="ExternalInput").ap()
    out_d = nc.dram_tensor("out", [BSH, IN], F32, kind="ExternalOutput").ap()
    if debug:
        dbg_hT = nc.dram_tensor("dbg_hT", [BSH, HID], FP16, kind="ExternalOutput").ap()
        dbg_h0 = nc.dram_tensor("dbg_h0", [128, 64], FP16, kind="ExternalOutput").ap()

    NT0 = HID // 128  # 4 o-tiles in layer 0
    NT1 = IN // 128   # 8 o-tiles in layer 1
    NG0 = BSH // BG0  # 8 batch groups in layer 0
    NG1 = BSH // BG1  # 4 batch groups in layer 1

    with TileContext(nc) as tc:
        with (
            tc.tile_pool(name="const", bufs=1) as consts,
            tc.tile_pool(name="vpool", bufs=2) as vpool,
            tc.tile_pool(name="psum", bufs=2, space="PSUM") as pp,
        ):
            ident = consts.tile([128, 128], FP16, tag="ident")
            make_identity(nc, ident[:])

            w0_sb = consts.tile([128, NCH0 * HID], FP16, tag="w0")
            nc.sync.dma_start(out=w0_sb[:], in_=w0_d[:, :])
            w1_sb = consts.tile([128, NCH1 * IN], FP16, tag="w1")
            nc.sync.dma_start(out=w1_sb[:], in_=w1_d[:, :])
            # one moving-matrix set per batch group, x rows prefetched so no
            # DMA sits in the matmul critical path
            rhs0 = []
            for s in range(BSH // BG0):
                r0 = consts.tile([128, BG0 * IN], FP16, tag=f"rhs0_{s}")
                nc.sync.dma_start(out=r0[:], in_=sel0_d[:, :])
                nc.sync.dma_start(
                    out=r0[127:128, :],
                    in_=x_d[0:1, s * BG0 * IN:(s + 1) * BG0 * IN])
                rhs0.append(r0)
            rhs1 = []
            for s in range(BSH // BG1):
                r1 = consts.tile([128, BG1 * HID], FP16, tag=f"rhs1_{s}")
                nc.sync.dma_start(out=r1[:], in_=sel1_d[:, :])
                rhs1.append(r1)
            sb0 = consts.tile([128, 8], F32, tag="sb0")
            nc.sync.dma_start(out=sb0[:], in_=sb0_d[:, :])
            sb1 = consts.tile([128, 16], F32, tag="sb1")
            nc.sync.dma_start(out=sb1[:], in_=sb1_d[:, :])

            h0 = consts.tile([128, NT0 * BSH], FP16, tag="h0")
            h0f = consts.tile([128, NT0 * BSH], FP16, tag="h0f")
            h1 = consts.tile([128, NT1 * BSH], FP16, tag="h1")
            h1f = consts.tile([128, NT1 * BSH], FP16, tag="h1f")
            hT = consts.tile([BSH, HID], FP16, tag="hT")
            hrow = consts.tile([1, BSH * HID], FP16, tag="hrow")
            out_sb = consts.tile([BSH, IN], F32, tag="out_sb")

            def layer(nt, ng, bg, in_f, nch, rhs_sets, w_sb, h_out, out_f,
                      pool_tiles=()):
                for ot in range(nt):
                    V = vpool.tile([128, BSH * in_f], FP16, tag="V")
                    for g in range(ng):
                        rset = rhs_sets[g]
                        px = pp.tile([128, bg * in_f], F32, tag="ps")
                        pxv = px[:].rearrange("p (b i) -> p b i", b=bg)
                        rv = rset[:].rearrange("p (b i) -> p b i", b=bg)
                        for k in range(nch):
                            i0, i1 = k * CL, min((k + 1) * CL, in_f)
                            lhsT = w_sb[:, k * out_f + ot * 128:
                                        k * out_f + ot * 128 + 128]
                            # one matmul per (batch row, PSUM-bank segment):
                            # a matmul output may not cross a 512-f32 bank
                            for bi in range(bg):
                                p0 = i0
                                while p0 < i1:
                                    p1 = min(i1, (p0 // 512 + 1) * 512)
                                    nc.tensor.matmul(
                                        pxv[:, bi:bi + 1, p0:p1],
                                        lhsT=lhsT,
                                        rhs=rv[:, bi:bi + 1, p0:p1],
                                        start=True,
                                        stop=True,
                                    )
                                    p0 = p1
                        nc.scalar.activation(
                            V[:, g * bg * in_f:(g + 1) * bg * in_f], px[:],
                            AF.Copy)
                    # in-place max tree over i: V [128, BSH, in_f] -> [:, :, 0:1]
                    eng = nc.gpsimd if ot in pool_tiles else nc.vector
                    lvl = V[:].rearrange("p (b i) -> p b i", b=BSH)
                    n = in_f
                    while n > 2:
                        eng.tensor_tensor(
                            out=lvl[:, :, 0:n // 2], in0=lvl[:, :, 0:n // 2],
                            in1=lvl[:, :, n // 2:n], op=ALU.max)
                        n //= 2
                    eng.tensor_tensor(
                        out=h_out[:, ot * BSH:(ot + 1) * BSH].rearrange(
                            "p (b i) -> p b i", b=BSH),
                        in0=lvl[:, :, 0:1], in1=lvl[:, :, 1:2], op=ALU.max)
                    eng.tensor_scalar_max(
                        h_out[:, ot * BSH:(ot + 1) * BSH],
                        h_out[:, ot * BSH:(ot + 1) * BSH], 0.0)

            # ---------------- layer 0 ----------------
            layer(NT0, NG0, BG0, IN, NCH0, rhs0, w0_sb, h0, HID,
                  pool_tiles=())
            for ot in range(NT0):
                nc.scalar.activation(
                    h0f[:, ot * BSH:(ot + 1) * BSH],
                    h0[:, ot * BSH:(ot + 1) * BSH],
                    AF.Identity,
                    bias=sb0[:, 2 * ot + 1: 2 * ot + 2],
                    scale=sb0[:, 2 * ot: 2 * ot + 1],
                )
            # h [512, 16] -> hT [16, 512] -> hrow [1, 8192]
            pt = pp.tile([BSH, HID], FP16, tag="ps")
            for ot in range(NT0):
                nc.tensor.transpose(
                    pt[:, ot * 128:(ot + 1) * 128],
                    h0f[:, ot * BSH:(ot + 1) * BSH],
                    ident[:],
                )
            nc.scalar.activation(hT[:], pt[:], AF.Copy)
            nc.sync.dma_start(out=hrow[:], in_=hT[:])
            if debug:
                nc.sync.dma_start(out=dbg_hT[:, :], in_=hT[:])
                nc.sync.dma_start(out=dbg_h0[:, :], in_=h0[:])

            # ---------------- layer 1 ----------------
            # layer 1 x rows come from hrow once it exists
            for s in range(BSH // BG1):
                nc.sync.dma_start(
                    out=rhs1[s][127:128, :],
                    in_=hrow[0:1, s * BG1 * HID:(s + 1) * BG1 * HID])
            layer(NT1, NG1, BG1, HID, NCH1, rhs1, w1_sb, h1, IN,
                  pool_tiles=())
            for ot in range(NT1):
                nc.scalar.activation(
                    h1f[:, ot * BSH:(ot + 1) * BSH],
                    h1[:, ot * BSH:(ot + 1) * BSH],
                    AF.Identity,
                    bias=sb1[:, 2 * ot + 1: 2 * ot + 2],
                    scale=sb1[:, 2 * ot: 2 * ot + 1],
                )
            po = pp.tile([BSH, IN], FP16, tag="ps")
            for ot in range(NT1):
                nc.tensor.transpose(
                    po[:, ot * 128:(ot + 1) * 128],
                    h1f[:, ot * BSH:(ot + 1) * BSH],
                    ident[:],
                )
            nc.scalar.activation(out_sb[:], po[:], AF.Copy)
            nc.sync.dma_start(out=out_d[:, :], in_=out_sb[:])
    nc.finalize()
    return nc


def kernel(**inputs):
    from concourse import bass_utils

    x = np.asarray(inputs["x"], dtype=np.float32)
    opmin0, mask0 = _selection(inputs["otc0"], inputs["etc0"], 0)
    opmin1, mask1 = _selection(inputs["otc1"], inputs["etc1"], 1)
    w0, sel0, sb0 = _prep_layer(opmin0, mask0)
    w1, sel1, sb1 = _prep_layer(opmin1, mask1)

    if "nc" not in _PROGRAM_CACHE:
        _PROGRAM_CACHE["nc"] = _build_program()
    nc = _PROGRAM_CACHE["nc"]

    x16 = x.astype(np.float16)
    in_maps = []
    for c in range(NCORES):
        in_maps.append({
            "x": x16[c * BSH:(c + 1) * BSH].reshape(1, -1),
            "w0": w0,
            "w1": w1,
            "sel0": sel0,
            "sel1": sel1,
            "sb0": sb0,
            "sb1": sb1,
        })
    res = bass_utils.run_bass_kernel_spmd(nc, in_maps, core_ids=list(range(NCORES)))
    out = np.concatenate([r["out"] for r in res.results], axis=0)
    return out.astype(np.float32)
